# revision 56
# baseline (speedup 1.0000x reference)
"""Trainium2 8-core Bass kernel for the UniGAT hypergraph attention block.

Algorithm (matches the jax reference numerically, up to bf16 rounding):
  1. Xh = X @ theta_cat + b          (per-core node shard, PE matmul)
  2. v2e: esum[e] = sum over incidence pairs (e,v) of Xh[v]
       - per-core partial over its node shard: dma_gather of Xh rows per
         pair (sorted by edge) + 0/1-indicator segment matmul on PE
       - AllReduce(esum) over the 8 cores
  3. Softmax folding: w = exp(s)/sum(exp(s)) exactly (the segment-max
     subtraction cancels; s = leaky_relu in [-0.5, 0.5] so exp is safe).
     Build per-edge table Z = [Y*expS | expS] where Y = esum*inv_cnt,
     expS[e,h] = exp(leaky_relu(inv_cnt*(esum @ aw_h))).
  4. e2v: plain 0/1 segment-sum of gathered Z rows per destination vertex
     (sorted by vertex) -> numerator (256 cols) and denominator (4 cols);
     divide per head.
  5. ELU -> LayerNorm -> GELU -> conv matmul -> X + gamma * Xo.

Sharding: nodes (and pairs grouped by destination vertex) across 8 cores;
weights and edge tables replicated; one AllReduce of esum is the only
collective.
"""

import os

import numpy as np
import ml_dtypes

import concourse.bass as bass
import concourse.bacc as bacc
import concourse.tile as tile
import concourse.mybir as mybir
from concourse.bass_utils import run_bass_kernel_spmd
from concourse.library_config import mlp

BF16 = mybir.dt.bfloat16
F8 = mybir.dt.float8e4
F32 = mybir.dt.float32
I16 = mybir.dt.int16
AL = mybir.AluOpType
AF = mybir.ActivationFunctionType

P = 128
NCORES = 8

N_NODES = 100000
N_EDGES = 20000
NNZ = 500000
CH = 256
H = 4
DH = 64
NEG_SLOPE = 0.2
LN_EPS = 1e-6

NPC = N_NODES // NCORES          # 12500
VG = (NPC + P - 1) // P          # 98
NPC_PAD = VG * P                 # 12544
EG = (N_EDGES + P - 1) // P      # 157
E_PAD = EG * P                   # 20096

ZW = 512                         # Z table row stride in fp8 bytes (260 used)
GATHER_CALL_V2E = 4096
GATHER_CALL_E2V = 4096
SELW = 16                        # indicator chunks built per DVE op


def _bf(x):
    return np.asarray(x, dtype=ml_dtypes.bfloat16)


def _f8(x):
    return np.asarray(x, dtype=ml_dtypes.float8_e4m3)


def _wrap16(idx):
    """dma_gather index layout: index i -> [i % 16, i // 16], replicated x8."""
    assert idx.size % 16 == 0
    w = idx.reshape(-1, 16).T
    return np.ascontiguousarray(np.tile(w, (8, 1))).astype(np.int16)


def _pairmajor(vals, dtype):
    """pair i -> [i % 128, i // 128]."""
    assert vals.size % P == 0
    return np.ascontiguousarray(vals.reshape(-1, P).T).astype(dtype)


def _sel_bytes(rel):
    """rel [M, P] (0..127 valid, 255 pad) -> fp8 one-hot sel tensor
    [P(slot), M*P] with [p, m*128+d] = (rel[m, p] == d)."""
    M = rel.shape[0]
    sel = np.zeros((M, P, P), dtype=ml_dtypes.float8_e4m3)
    m_i, p_i = np.nonzero(rel != 255.0)
    sel[m_i, p_i, rel[m_i, p_i].astype(np.int64)] = 1.0
    return np.ascontiguousarray(sel.transpose(1, 0, 2)).reshape(P, M * P)


def _balance_groups(deg):
    """Assign NPC vertices to VG groups of <=128, balancing pair sums.
    Returns perm: old local id -> new local id."""
    import heapq
    order = np.argsort(-deg, kind="stable")
    heap = [(0, 0, g) for g in range(VG)]
    heapq.heapify(heap)
    perm = np.zeros(NPC, dtype=np.int64)
    fill = np.zeros(VG, dtype=np.int64)
    for v in order:
        while True:
            s, cnt, g = heapq.heappop(heap)
            if cnt < P:
                perm[v] = g * P + fill[g]
                fill[g] += 1
                heapq.heappush(heap, (s + int(deg[v]), cnt + 1, g))
                break
    return perm


def _build_streams_unaligned(vals, rels, cnts):
    """Common unaligned layout: group g occupies slots [S_g, S_g+cap_g),
    cap_g = max over cores. Returns idx [NCORES,T], rel columns per matmul
    entry [NCORES, M, P], and per-group entry chunk lists."""
    ngroups = cnts.shape[1]
    caps = np.maximum(cnts.max(axis=0), 1)
    starts = np.concatenate([[0], np.cumsum(caps)])
    T = int(-(-starts[-1] // P) * P)
    entries = []            # (g, k) in emission order (g-major)
    group_ks = []
    for g in range(ngroups):
        k0 = int(starts[g] // P)
        k1 = int((starts[g + 1] - 1) // P)
        ks = list(range(k0, k1 + 1))
        group_ks.append(ks)
        entries.extend((g, k) for k in ks)
    M = len(entries)
    idx_s = np.zeros((NCORES, T), dtype=np.int64)
    rel_s = np.full((NCORES, M, P), 255.0, dtype=np.float32)
    for c in range(NCORES):
        gstart = np.concatenate([[0], np.cumsum(cnts[c])])
        for g in range(ngroups):
            n = int(cnts[c][g])
            if n == 0:
                continue
            s, d = int(gstart[g]), int(starts[g])
            idx_s[c, d:d + n] = vals[c][s:s + n]
        for m, (g, k) in enumerate(entries):
            lo = max(int(starts[g]), k * P)
            hi = min(int(starts[g]) + int(cnts[c][g]), (k + 1) * P)
            if hi <= lo:
                continue
            s = int(gstart[g]) + (lo - int(starts[g]))
            rel_s[c, m, lo - k * P:hi - k * P] = rels[c][s:s + hi - lo] % P
    return idx_s, rel_s, group_ks, T, M


def make_plan(edge_idx, vertex_idx):
    """Host-side index preprocessing (graph structure only)."""
    edge_idx = np.asarray(edge_idx).astype(np.int64)
    vertex_idx = np.asarray(vertex_idx).astype(np.int64)
    core = vertex_idx // NPC
    lv = vertex_idx - core * NPC

    v2e_e, v2e_lv = [], []
    e2v_e, e2v_nlv = [], []
    perms = []
    for c in range(NCORES):
        m = core == c
        e_c, lv_c = edge_idx[m], lv[m]
        o = np.argsort(e_c, kind="stable")
        v2e_e.append(e_c[o])
        v2e_lv.append(lv_c[o])
        deg = np.bincount(lv_c, minlength=NPC)
        perm = _balance_groups(deg)
        perms.append(perm)
        nlv_c = perm[lv_c]
        o = np.argsort(nlv_c, kind="stable")
        e2v_e.append(e_c[o])
        e2v_nlv.append(nlv_c[o])

    def group_counts(keys_list, ngroups):
        cnts = np.zeros((NCORES, ngroups), dtype=np.int64)
        for c in range(NCORES):
            cnts[c] = np.bincount(keys_list[c] // P, minlength=ngroups)
        return cnts

    v2e_cnts = group_counts(v2e_e, EG)
    e2v_cnts = group_counts(e2v_nlv, VG)

    v2e_idx, v2e_rel, v2e_gks, v2e_T, v2e_M = _build_streams_unaligned(
        v2e_lv, v2e_e, v2e_cnts)
    e2v_idx, e2v_rel, e2v_gks, e2v_T, e2v_M = _build_streams_unaligned(
        e2v_e, e2v_nlv, e2v_cnts)

    cnt = np.bincount(edge_idx, minlength=E_PAD).astype(np.float32)
    inv_cnt = 1.0 / np.maximum(cnt, 1.0)

    return dict(
        v2e_gks=v2e_gks, e2v_gks=e2v_gks,
        v2e_T=v2e_T, e2v_T=e2v_T, v2e_M=v2e_M, e2v_M=e2v_M,
        v2e_idx=v2e_idx, v2e_rel=v2e_rel,
        e2v_idx=e2v_idx, e2v_rel=e2v_rel,
        perms=perms,
        inv_cnt=inv_cnt,
    )


def _n_gather_calls(total_chunks, call_pairs):
    total = total_chunks * P
    n_full, rem = divmod(total, call_pairs)
    sizes = [call_pairs] * n_full
    if rem:
        sizes.append(rem)
    return sizes


def build_kernel(v2e_gks, e2v_gks, v2e_T, e2v_T, v2e_M, e2v_M,
                 debug_tables=False):
    v2e_tot = v2e_T
    e2v_tot = e2v_T

    nc = bacc.Bacc("TRN2", target_bir_lowering=False, debug=False,
                   num_devices=NCORES, num_swdge_queues=2,
                   dynamic_dma_scratch_size=32768)

    x_in = nc.dram_tensor("x", [NPC_PAD, CH], BF16, kind="ExternalInput")
    xt_in = nc.dram_tensor("xt", [P, VG * 2 * P], BF16, kind="ExternalInput")
    wcat_in = nc.dram_tensor("wcat", [CH, CH], BF16, kind="ExternalInput")
    convw_in = nc.dram_tensor("convw", [CH, CH], BF16, kind="ExternalInput")
    browbf_in = nc.dram_tensor("browbf", [1, CH], BF16, kind="ExternalInput")
    awrep_in = nc.dram_tensor("awrep", [P, CH], BF16, kind="ExternalInput")
    convbrep_in = nc.dram_tensor("convbrep", [P, CH], F32, kind="ExternalInput")
    gammarep_in = nc.dram_tensor("gammarep", [P, CH], F32, kind="ExternalInput")
    lnwrep_in = nc.dram_tensor("lnwrep", [P, CH], F32, kind="ExternalInput")
    lnbrep_in = nc.dram_tensor("lnbrep", [P, CH], F32, kind="ExternalInput")
    iota_in = nc.dram_tensor("iota", [P, P], BF16, kind="ExternalInput")
    ident_in = nc.dram_tensor("ident", [P, P], BF16, kind="ExternalInput")
    invc_in = nc.dram_tensor("invc", [P, EG], F32, kind="ExternalInput")
    c14_in = nc.dram_tensor("c14", [P, H], F32, kind="ExternalInput")
    epscol_in = nc.dram_tensor("epscol", [P, 1], F32, kind="ExternalInput")
    v2ei_in = nc.dram_tensor("v2ei", [P, v2e_tot // 16], I16, kind="ExternalInput")
    v2er_in = nc.dram_tensor("v2er", [P, v2e_M], BF16, kind="ExternalInput")
    e2vi_in = nc.dram_tensor("e2vi", [P, e2v_tot // 16], I16, kind="ExternalInput")
    e2vr_in = nc.dram_tensor("e2vr", [P, e2v_M], BF16, kind="ExternalInput")
    # f32: the gamma-scaled (1e-6) GNN term is below bf16 resolution vs X,
    # so a bf16 output would silently drop it.
    out_ext = nc.dram_tensor("out", [NPC_PAD, CH], F32, kind="ExternalOutput")
    dbg = {}
    if debug_tables:
        dbg["xh"] = nc.dram_tensor("dbg_xh", [NPC_PAD, CH], F8, kind="ExternalOutput")
        dbg["esum"] = nc.dram_tensor("dbg_esum", [E_PAD, CH], BF16, kind="ExternalOutput")
        dbg["z"] = nc.dram_tensor("dbg_z", [E_PAD, ZW], F8, kind="ExternalOutput")
        dbg["xn"] = nc.dram_tensor("dbg_xn", [NPC_PAD, CH], BF16, kind="ExternalOutput")

    def rows(dr, t0, w):
        return dr[t0 * P:(t0 + w) * P, :].rearrange("(t p) f -> p t f", p=P)

    with tile.TileContext(nc) as tc:
        with tc.tile_pool(name="dram", bufs=1, space="DRAM") as dram, \
             tc.tile_pool(name="const", bufs=1) as cpool, \
             tc.tile_pool(name="resident", bufs=1) as rpool:

            nc.gpsimd.load_library(mlp)

            xh_table = dram.tile([NPC_PAD, CH], F8)
            esum_bounce = dram.tile([E_PAD, CH], BF16)
            # non-final bounds must be multiples of 4: esum is evacuated in
            # 4-group batches and the AR read races a not-yet-emitted write
            # otherwise.
            AR_BOUNDS = [36, 84, 120, 148, EG]
            _ar_lims = list(zip([0] + AR_BOUNDS[:-1], AR_BOUNDS))
            yfulls = []
            for _ci, (_a, _b) in enumerate(_ar_lims):
                yf = dram.tile([(_b - _a) * P, CH], BF16, addr_space="Shared",
                               name=f"yfull{_ci}", tag=f"yfull{_ci}")
                yfulls.append(yf)
            z_table = dram.tile([E_PAD, ZW], F8)

            def yrows(t0, w):
                """rows [t0*128,(t0+w)*128) of the chunked AR output; the
                caller must not cross an AR chunk boundary."""
                for (_a, _b), yf in zip(_ar_lims, yfulls):
                    if t0 >= _a and t0 + w <= _b:
                        return yf[(t0 - _a) * P:(t0 - _a + w) * P, :].rearrange(
                            "(t p) f -> p t f", p=P)
                raise AssertionError("yrows crosses AR chunk")

            def cload(dr, shape, dtype, name):
                t = cpool.tile(shape, dtype, name=name, tag=name)
                nc.sync.dma_start(t[:], dr[:])
                return t

            w_sb = cpool.tile([P, 2, CH], BF16)
            nc.sync.dma_start(w_sb[:], wcat_in[:].rearrange("(k p) f -> p k f", p=P))
            convw_sb = cpool.tile([P, 2, CH], BF16)
            nc.sync.dma_start(convw_sb[:], convw_in[:].rearrange("(k p) f -> p k f", p=P))
            browbf = cload(browbf_in, [1, CH], BF16, "browbf")
            ones1 = cpool.tile([1, P], BF16)
            nc.vector.memset(ones1[:], 1.0)
            awrep = cload(awrep_in, [P, CH], BF16, "awrep")
            convbrep = cload(convbrep_in, [P, CH], F32, "convbrep")
            gammarep = cload(gammarep_in, [P, CH], F32, "gammarep")
            lnwrep = cload(lnwrep_in, [P, CH], F32, "lnwrep")
            lnbrep = cload(lnbrep_in, [P, CH], F32, "lnbrep")
            iota = cload(iota_in, [P, P], BF16, "iota")
            ident = cload(ident_in, [P, P], BF16, "ident")
            invc = cload(invc_in, [P, EG], F32, "invc")
            c14 = cload(c14_in, [P, H], F32, "c14")
            epscol = cload(epscol_in, [P, 1], F32, "epscol")
            gcb = cpool.tile([P, CH], F32)
            nc.vector.tensor_tensor(out=gcb[:], in0=gammarep[:], in1=convbrep[:],
                                    op=AL.mult)
            # Warmup collective: the first AllReduce pays ~150us of CC mesh
            # init; absorb it here, overlapped with phase 1.
            warm_in = dram.tile([P, CH], BF16)
            warm_out = dram.tile([P, CH], BF16, addr_space="Shared",
                                 name="warmout", tag="warmout")
            wt = cpool.tile([P, CH], BF16)
            nc.vector.memset(wt[:], 0.0)
            nc.sync.dma_start(warm_in[:], wt[:])
            nc.gpsimd.collective_compute(
                "AllReduce", AL.add, replica_groups=[list(range(NCORES))],
                ins=[warm_in[:].opt()], outs=[warm_out[:].opt()])
            # fold gamma into conv_w columns: (Xg @ W) * gamma = Xg @ (W * gamma_row)
            nc.vector.tensor_tensor(
                out=convw_sb[:], in0=convw_sb[:],
                in1=gammarep[:, None, :].to_broadcast([P, 2, CH]), op=AL.mult)

            def z_blocks(zpool, t_lo, t_hi):
                for t0 in range(t_lo, t_hi, 8):
                    w = min(8, t_hi - t0)
                    y4 = zpool.tile([P, 8, CH], BF16, tag="zy")
                    nc.sync.dma_start(y4[:, :w, :], yrows(t0, w))
                    tmp = zpool.tile([P, 8, CH], BF16, tag="ztmp")
                    nc.vector.tensor_tensor(
                        out=tmp[:, :w, :], in0=y4[:, :w, :],
                        in1=awrep[:, None, :].to_broadcast([P, w, CH]), op=AL.mult)
                    beta = zpool.tile([P, 8, H], F32, tag="zbeta")
                    nc.vector.tensor_reduce(
                        out=beta[:, :w, :],
                        in_=tmp[:, :w, :].rearrange("p t (h d) -> p t h d", d=DH),
                        axis=mybir.AxisListType.X, op=AL.add)
                    al_ = zpool.tile([P, 8, H], F32, tag="zal")
                    nc.vector.tensor_tensor(
                        out=al_[:, :w, :], in0=beta[:, :w, :],
                        in1=invc[:, t0:t0 + w, None].to_broadcast([P, w, H]),
                        op=AL.mult)
                    sal = zpool.tile([P, 8, H], F32, tag="zsal")
                    nc.scalar.activation(out=sal[:, :w, :], in_=al_[:, :w, :],
                                         func=AF.Prelu, alpha=NEG_SLOPE)
                    zrow = zpool.tile([P, 8, CH + H], F8, tag="zrow")
                    expS = zrow[:, :w, CH:CH + H]
                    nc.scalar.activation(out=expS, in_=sal[:, :w, :], func=AF.Exp)
                    s4 = zpool.tile([P, 8, H], F32, tag="zs4")
                    nc.vector.tensor_tensor(
                        out=s4[:, :w, :], in0=expS,
                        in1=invc[:, t0:t0 + w, None].to_broadcast([P, w, H]),
                        op=AL.mult)
                    nc.vector.tensor_tensor(
                        out=zrow[:, :w, :CH].rearrange("p t (h d) -> p t h d", d=DH),
                        in0=y4[:, :w, :].rearrange("p t (h d) -> p t h d", d=DH),
                        in1=s4[:, :w, :, None].to_broadcast([P, w, H, DH]),
                        op=AL.mult)
                    nc.sync.dma_start(
                        z_table[t0 * P:(t0 + w) * P, :CH + H].rearrange(
                            "(t p) f -> p t f", p=P),
                        zrow[:, :w, :])


            # ================= Phase 1: Xh = X @ W + b =================
            with tc.tile_pool(name="p1sb", bufs=3) as p1sb, \
                 tc.tile_pool(name="p1xt", bufs=1) as p1xt, \
                 tc.tile_pool(name="p1ps", bufs=2, space="PSUM") as p1ps:
                xt_sb = p1xt.tile([P, VG * 2 * P], BF16, tag="xt")
                XTC = VG * 2 * P // 4
                assert XTC * 4 == VG * 2 * P
                for c4 in range(4):
                    nc.sync.dma_start(xt_sb[:, c4 * XTC:(c4 + 1) * XTC],
                                      xt_in[:, c4 * XTC:(c4 + 1) * XTC])
                xt_v = xt_sb[:].rearrange("p (t k f) -> p t k f", t=VG, k=2)
                xh4, t0, tw = None, 0, 0
                for t in range(VG):
                    psf = p1ps.tile([P, 512], F32, tag="xhps")
                    ps = psf[:, :CH]
                    for k in range(2):
                        nc.tensor.matmul(ps, lhsT=xt_v[:, t, k, :], rhs=w_sb[:, k, :],
                                         start=(k == 0), stop=False)
                    nc.tensor.matmul(ps, lhsT=ones1[:], rhs=browbf[:],
                                     start=False, stop=True)
                    if t % 4 == 0:
                        t0 = t
                        tw = min(4, VG - t0)
                        xh4 = p1sb.tile([P, 4, CH], F8, tag="xhout")
                    nc.scalar.copy(out=xh4[:, t - t0, :], in_=ps)
                    if t - t0 == tw - 1:
                        nc.sync.dma_start(rows(xh_table, t0, tw), xh4[:, :tw, :])
                if debug_tables:
                    nc.sync.dma_start(dbg["xh"][:], xh_table[:])

            # ================= Phase 2: v2e partial esum =================
            with tc.tile_pool(name="v2esb", bufs=4) as gpool, \
                 tc.tile_pool(name="v2esel", bufs=3) as selpool, \
                 tc.tile_pool(name="v2eev", bufs=3) as evpool, \
                 tc.tile_pool(name="v2eidx", bufs=1) as ipool, \
                 tc.tile_pool(name="zsbv", bufs=3) as zpool_v2e, \
                 tc.tile_pool(name="v2eps", bufs=4, space="PSUM") as v2eps:
                v2ei = ipool.tile([P, v2e_tot // 16], I16)
                nc.sync.dma_start(v2ei[:], v2ei_in[:])
                v2er = ipool.tile([P, v2e_M], BF16)
                nc.sync.dma_start(v2er[:], v2er_in[:])
                call_sizes = _n_gather_calls(v2e_tot // P, GATHER_CALL_V2E)
                gtiles = [None] * len(call_sizes)
                sel_cur, sel0 = None, 0
                esb4, e0, ew = None, 0, 0
                ent = 0
                # z_blocks for AR chunk ci run LAG groups after its bound, in
                # fine-grained strips of 8 tiles to avoid DVE bursts.
                LAG = 26
                ZSTRIP = 8
                _zq = {}
                for _ci, (_a, _b) in enumerate(_ar_lims):
                    for _si, _t0 in enumerate(range(_a, _b, ZSTRIP)):
                        _zq.setdefault(_b + LAG + 3 * _si, []).append(
                            (_t0, min(_t0 + ZSTRIP, _b)))
                for g in range(EG):
                    for _a, _b in _zq.get(g, []):
                        z_blocks(zpool_v2e, _a, _b)
                    psf = v2eps.tile([P, 512], F32, tag="v2eps")
                    ps = psf[:, :CH]
                    ks = v2e_gks[g]
                    for i, k in enumerate(ks):
                        gc, j = divmod(k, GATHER_CALL_V2E // P)
                        if gtiles[gc] is None:
                            n = call_sizes[gc]
                            gt = gpool.tile([P, GATHER_CALL_V2E // P, CH], F8,
                                            tag="v2egather")
                            s = gc * GATHER_CALL_V2E
                            nc.gpsimd.dma_gather(
                                gt[:, :n // P, :], xh_table[:],
                                v2ei[:, s // 16:(s + n) // 16], n, n, CH,
                                single_packet=False, queue_num=gc % 2)
                            gtiles[gc] = gt
                        if ent % SELW == 0:
                            sel0 = ent
                            sw = min(SELW, v2e_M - ent)
                            sel_cur = selpool.tile([P, SELW, P], F8, tag="v2esel")
                            nc.vector.tensor_tensor(
                                out=sel_cur[:, :sw, :],
                                in0=v2er[:, ent:ent + sw, None].to_broadcast(
                                    [P, sw, P]),
                                in1=iota[:, None, :].to_broadcast([P, sw, P]),
                                op=AL.is_equal)
                        nc.tensor.matmul(ps, lhsT=sel_cur[:, ent - sel0, :],
                                         rhs=gtiles[gc][:, j, :],
                                         start=(i == 0), stop=(i == len(ks) - 1))
                        ent += 1
                    if g % 4 == 0:
                        e0 = g
                        ew = min(4, EG - e0)
                        esb4 = evpool.tile([P, 4, CH], BF16, tag="v2eev")
                    nc.scalar.copy(out=esb4[:, g - e0, :], in_=ps)
                    if g - e0 == ew - 1:
                        nc.sync.dma_start(rows(esum_bounce, e0, ew), esb4[:, :ew, :])
                    if g + 1 in AR_BOUNDS:
                        ci = AR_BOUNDS.index(g + 1)
                        a = 0 if ci == 0 else AR_BOUNDS[ci - 1]
                        nc.gpsimd.collective_compute(
                            "AllReduce", AL.add,
                            replica_groups=[list(range(NCORES))],
                            ins=[esum_bounce[a * P:(g + 1) * P, :].opt()],
                            outs=[yfulls[ci].opt()])

                for g in range(EG, EG + 96):
                    for _a, _b in _zq.get(g, []):
                        z_blocks(zpool_v2e, _a, _b)

            # ================= Phase 3: AllReduce (issued chunked in phase 2)
            if debug_tables:
                for (_a, _b), yf in zip(_ar_lims, yfulls):
                    nc.sync.dma_start(dbg["esum"][_a * P:_b * P, :], yf[:])

            if debug_tables:
                with tc.tile_pool(name="dbgz", bufs=1) as _dzp:
                    nc.sync.dma_start(dbg["z"][:], z_table[:])

            # ================= Phase 5: e2v + ELU + LN =================
            with tc.tile_pool(name="e2vsb", bufs=3) as gpool2, \
                 tc.tile_pool(name="e2vsel", bufs=3) as selpool2, \
                 tc.tile_pool(name="e2vev", bufs=2) as evpool2, \
                 tc.tile_pool(name="e2vidx", bufs=1) as ipool2, \
                 tc.tile_pool(name="fsb", bufs=3) as fpool, \
                 tc.tile_pool(name="fps", bufs=2, space="PSUM") as fps, \
                 tc.tile_pool(name="ftps", bufs=2, space="PSUM") as ftps, \
                 tc.tile_pool(name="e2vps", bufs=3, space="PSUM") as e2vps:
                e2vi = ipool2.tile([P, e2v_tot // 16], I16)
                nc.sync.dma_start(e2vi[:], e2vi_in[:])
                e2vr = ipool2.tile([P, e2v_M], BF16)
                nc.sync.dma_start(e2vr[:], e2vr_in[:])
                call_sizes = _n_gather_calls(e2v_tot // P, GATHER_CALL_E2V)
                gtiles = [None] * len(call_sizes)
                sel_cur, sel0 = None, 0
                elu4, l0, lw = None, 0, 0
                ent = 0
                xc_buf = ipool2.tile([P, VG, CH], BF16)
                var_buf = ipool2.tile([P, VG], F32)
                _PB_BOUNDS = [16, 32, 48, 64, 80, 88, 96]

                def pass_b(lo, hi):
                    vb = evpool2.tile([P, 16], F32, tag="vbeps")
                    nc.vector.tensor_scalar_add(vb[:, :hi - lo],
                                                var_buf[:, lo:hi], LN_EPS)
                    vrec = evpool2.tile([P, 16], F32, tag="vrec")
                    nc.vector.reciprocal(vrec[:, :hi - lo], vb[:, :hi - lo])
                    rstd = evpool2.tile([P, 16], F32, tag="rstd")
                    nc.scalar.activation(out=rstd[:, :hi - lo],
                                         in_=vrec[:, :hi - lo], func=AF.Sqrt)
                    for b0 in range(lo, hi, 4):
                        bw = min(4, hi - b0)
                        t4 = evpool2.tile([P, 4, CH], BF16, tag="t4")
                        nc.vector.tensor_tensor(
                            out=t4[:, :bw, :], in0=xc_buf[:, b0:b0 + bw, :],
                            in1=rstd[:, b0 - lo:b0 - lo + bw, None].to_broadcast(
                                [P, bw, CH]),
                            op=AL.mult)
                        t5 = evpool2.tile([P, 4, CH], F32, tag="t5")
                        nc.vector.tensor_tensor(
                            out=t5[:, :bw, :], in0=t4[:, :bw, :],
                            in1=lnwrep[:, None, :].to_broadcast([P, bw, CH]),
                            op=AL.mult)
                        xnb = evpool2.tile([P, 4, CH], BF16, tag="xnb")
                        nc.vector.tensor_tensor(
                            out=xnb[:, :bw, :], in0=t5[:, :bw, :],
                            in1=lnbrep[:, None, :].to_broadcast([P, bw, CH]),
                            op=AL.add)
                        if debug_tables:
                            nc.sync.dma_start(rows(dbg["xn"], b0, bw),
                                              xnb[:, :bw, :])
                        xg4 = fpool.tile([P, 4, CH], BF16, tag="xg4")
                        nc.scalar.activation(out=xg4[:, :bw, :], in_=xnb[:, :bw, :],
                                             func=AF.Gelu)
                        x4 = fpool.tile([P, 4, CH], BF16, tag="x4")
                        nc.sync.dma_start(x4[:, :bw, :], rows(x_in, b0, bw))
                        xgc4 = fpool.tile([P, 4, CH], F32, tag="xgc4")
                        nc.vector.tensor_tensor(
                            out=xgc4[:, :bw, :], in0=x4[:, :bw, :],
                            in1=gcb[:, None, :].to_broadcast([P, bw, CH]),
                            op=AL.add)
                        ofin4 = fpool.tile([P, 4, CH], F32, tag="ofin4")
                        for j in range(bw):
                            xgT = fpool.tile([P, 2, P], BF16, tag="xgT")
                            for k in range(2):
                                tp = ftps.tile([P, P], BF16, tag="tps")
                                nc.tensor.transpose(tp[:],
                                                    xg4[:, j, k * P:(k + 1) * P],
                                                    ident[:])
                                nc.scalar.copy(out=xgT[:, k, :], in_=tp[:])
                            psf2 = fps.tile([P, 512], F32, tag="fps")
                            ps2 = psf2[:, :CH]
                            for k in range(2):
                                nc.tensor.matmul(ps2, lhsT=xgT[:, k, :],
                                                 rhs=convw_sb[:, k, :],
                                                 start=(k == 0), stop=(k == 1))
                            nc.vector.tensor_tensor(out=ofin4[:, j, :], in0=ps2,
                                                    in1=xgc4[:, j, :], op=AL.add)
                        nc.sync.dma_start(rows(out_ext, b0, bw), ofin4[:, :bw, :])

                for g in range(VG):
                    psf = e2vps.tile([P, 512], F32, tag="e2vps")
                    ps = psf[:, :CH + H]
                    ks = e2v_gks[g]
                    for i, k in enumerate(ks):
                        gc, j = divmod(k, GATHER_CALL_E2V // P)
                        if gtiles[gc] is None:
                            n = call_sizes[gc]
                            gt = gpool2.tile([P, GATHER_CALL_E2V // P, ZW], F8,
                                             tag="e2vgather")
                            s = gc * GATHER_CALL_E2V
                            nc.gpsimd.dma_gather(
                                gt[:, :n // P, :], z_table[:],
                                e2vi[:, s // 16:(s + n) // 16], n, n, ZW,
                                single_packet=False, queue_num=gc % 2)
                            gtiles[gc] = gt
                        if ent % SELW == 0:
                            sel0 = ent
                            sw = min(SELW, e2v_M - ent)
                            sel_cur = selpool2.tile([P, SELW, P], F8, tag="e2vsel")
                            nc.vector.tensor_tensor(
                                out=sel_cur[:, :sw, :],
                                in0=e2vr[:, ent:ent + sw, None].to_broadcast(
                                    [P, sw, P]),
                                in1=iota[:, None, :].to_broadcast([P, sw, P]),
                                op=AL.is_equal)
                        nc.tensor.matmul(ps, lhsT=sel_cur[:, ent - sel0, :],
                                         rhs=gtiles[gc][:, j, :CH + H],
                                         start=(i == 0), stop=(i == len(ks) - 1))
                        ent += 1
                    # xpre = num/den ; ELU = exp(min(x,0)) - 1 + relu(x)
                    den = evpool2.tile([P, H], F32, tag="den")
                    nc.vector.tensor_scalar_max(den[:], ps[:, CH:CH + H], 1e-12)
                    rec = evpool2.tile([P, H], F32, tag="rec")
                    nc.vector.reciprocal(rec[:], den[:])
                    xpre = evpool2.tile([P, CH], BF16, tag="xpre")
                    nc.vector.tensor_tensor(
                        out=xpre[:].rearrange("p (h d) -> p h d", d=DH),
                        in0=ps[:, :CH].rearrange("p (h d) -> p h d", d=DH),
                        in1=rec[:, :, None].to_broadcast([P, H, DH]),
                        op=AL.mult)
                    relx = evpool2.tile([P, CH], BF16, tag="relx")
                    nc.scalar.activation(out=relx[:], in_=xpre[:], func=AF.Relu)
                    m0 = evpool2.tile([P, CH], BF16, tag="m0")
                    nc.scalar.activation(out=m0[:], in_=xpre[:], func=AF.Relu,
                                         scale=-1.0)
                    ep = evpool2.tile([P, CH], F32, tag="ep")
                    nc.scalar.activation(out=ep[:], in_=m0[:], func=AF.Exp,
                                         scale=-1.0)
                    if g % 4 == 0:
                        l0 = g
                        lw = min(4, VG - l0)
                        elu4 = evpool2.tile([P, 4, CH], BF16, tag="elu4")
                    nc.vector.scalar_tensor_tensor(
                        out=elu4[:, g - l0, :], in0=ep[:], scalar=-1.0, in1=relx[:],
                        op0=AL.add, op1=AL.add)
                    if g - l0 == lw - 1:
                        mu4 = evpool2.tile([P, 4], F32, tag="mu4")
                        nc.vector.tensor_reduce(out=mu4[:, :lw], in_=elu4[:, :lw, :],
                                                axis=mybir.AxisListType.X, op=AL.add)
                        nc.vector.tensor_tensor(out=mu4[:, :lw], in0=mu4[:, :lw],
                                                in1=c14[:, :lw], op=AL.mult)
                        xc4 = xc_buf[:, l0:l0 + lw, :]
                        nc.vector.tensor_tensor(
                            out=xc4, in0=elu4[:, :lw, :],
                            in1=mu4[:, :lw, None].to_broadcast([P, lw, CH]),
                            op=AL.subtract)
                        sq4 = evpool2.tile([P, 4, CH], BF16, tag="sq4")
                        nc.vector.tensor_tensor(out=sq4[:, :lw, :], in0=xc4,
                                                in1=xc4, op=AL.mult)
                        ss4 = evpool2.tile([P, 4], F32, tag="ss4")
                        nc.vector.tensor_reduce(out=ss4[:, :lw], in_=sq4[:, :lw, :],
                                                axis=mybir.AxisListType.X, op=AL.add)
                        nc.vector.tensor_tensor(out=var_buf[:, l0:l0 + lw],
                                                in0=ss4[:, :lw],
                                                in1=c14[:, :lw], op=AL.mult)
                    if (g + 1) in _PB_BOUNDS:
                        pi = _PB_BOUNDS.index(g + 1)
                        pass_b(0 if pi == 0 else _PB_BOUNDS[pi - 1], g + 1)
                if _PB_BOUNDS[-1] < VG:
                    pass_b(_PB_BOUNDS[-1], VG)

    nc.compile()
    return nc


def prepare_inputs(X, edge_idx, vertex_idx, theta_w, theta_b, atten_w,
                   ln_w, ln_b, conv_w, conv_b, gamma, plan):
    X = np.asarray(X, dtype=np.float32)
    theta_w = np.asarray(theta_w, dtype=np.float32)
    wcat = _bf(theta_w.transpose(1, 0, 2).reshape(CH, CH))
    browbf = _bf(np.asarray(theta_b, np.float32).reshape(1, CH))
    awrep = _bf(np.tile(np.asarray(atten_w, np.float32).reshape(1, CH), (P, 1)))
    convw = _bf(np.asarray(conv_w, np.float32))
    convbrep = np.tile(np.asarray(conv_b, np.float32).reshape(1, CH), (P, 1))
    gammarep = np.tile(np.asarray(gamma, np.float32).reshape(1, CH), (P, 1))
    lnwrep = np.tile(np.asarray(ln_w, np.float32).reshape(1, CH), (P, 1))
    lnbrep = np.tile(np.asarray(ln_b, np.float32).reshape(1, CH), (P, 1))
    iota = _bf(np.tile(np.arange(P, dtype=np.float32), (P, 1)))
    ident = _bf(np.eye(P, dtype=np.float32))
    invc = np.ascontiguousarray(
        plan["inv_cnt"].reshape(EG, P).T).astype(np.float32)
    c14 = np.full((P, H), 1.0 / CH, np.float32)
    epscol = np.full((P, 1), LN_EPS, np.float32)

    in_maps = []
    for c in range(NCORES):
        xc = np.zeros((NPC_PAD, CH), np.float32)
        xc[:NPC] = X[c * NPC:(c + 1) * NPC]
        xcb = _bf(xc)
        # xt / xh_table stay in original local order (v2e gathers by old lv);
        # the residual input x follows the e2v output renumbering.
        xt = np.ascontiguousarray(
            _bf(xc).reshape(VG, P, 2, P).transpose(3, 0, 2, 1)).reshape(
                P, VG * 2 * P)
        perm = plan["perms"][c]
        xp = np.zeros((NPC_PAD, CH), np.float32)
        xp[perm] = xc[:NPC]
        in_maps.append(dict(
            x=_bf(xp), xt=xt, wcat=wcat, convw=convw,
            browbf=browbf, awrep=awrep,
            convbrep=convbrep.astype(np.float32),
            gammarep=gammarep.astype(np.float32),
            lnwrep=lnwrep.astype(np.float32), lnbrep=lnbrep.astype(np.float32),
            iota=iota, ident=ident, invc=invc, c14=c14, epscol=epscol,
            v2ei=_wrap16(plan["v2e_idx"][c]),
            v2er=np.ascontiguousarray(
                plan["v2e_rel"][c].T).astype(ml_dtypes.bfloat16),
            e2vi=_wrap16(plan["e2v_idx"][c]),
            e2vr=np.ascontiguousarray(
                plan["e2v_rel"][c].T).astype(ml_dtypes.bfloat16),
        ))
    return in_maps


_CACHE = {}


def kernel(X, edge_idx, vertex_idx, theta_w, theta_b, atten_w,
           ln_w, ln_b, conv_w, conv_b, gamma):
    debug_tables = bool(int(os.environ.get("GNN_DEBUG_TABLES", "0")))
    trace = bool(int(os.environ.get("GNN_TRACE", "0")))

    plan = make_plan(edge_idx, vertex_idx)
    key = (tuple(map(tuple, plan["v2e_gks"])), tuple(map(tuple, plan["e2v_gks"])),
           debug_tables)
    if key not in _CACHE:
        _CACHE[key] = build_kernel(plan["v2e_gks"], plan["e2v_gks"],
                                   plan["v2e_T"], plan["e2v_T"],
                                   plan["v2e_M"], plan["e2v_M"],
                                   debug_tables=debug_tables)
    nc = _CACHE[key]

    in_maps = prepare_inputs(X, edge_idx, vertex_idx, theta_w, theta_b,
                             atten_w, ln_w, ln_b, conv_w, conv_b, gamma, plan)
    res = run_bass_kernel_spmd(nc, in_maps, core_ids=list(range(NCORES)),
                               trace=trace)
    kernel.last_results = res
    outs = []
    for c in range(NCORES):
        o = np.asarray(res.results[c]["out"]).astype(np.float32)
        outs.append(o[plan["perms"][c]])
    return np.concatenate(outs, axis=0)



# revision 57
# speedup vs baseline: 1.1575x; 1.1575x over previous
"""Trainium2 8-core Bass kernel for the UniGAT hypergraph attention block.

Algorithm (matches the jax reference numerically, up to bf16 rounding):
  1. Xh = X @ theta_cat + b          (per-core node shard, PE matmul)
  2. v2e: esum[e] = sum over incidence pairs (e,v) of Xh[v]
       - per-core partial over its node shard: dma_gather of Xh rows per
         pair (sorted by edge) + 0/1-indicator segment matmul on PE
       - AllReduce(esum) over the 8 cores
  3. Softmax folding: w = exp(s)/sum(exp(s)) exactly (the segment-max
     subtraction cancels; s = leaky_relu in [-0.5, 0.5] so exp is safe).
     Build per-edge table Z = [Y*expS | expS] where Y = esum*inv_cnt,
     expS[e,h] = exp(leaky_relu(inv_cnt*(esum @ aw_h))).
  4. e2v: plain 0/1 segment-sum of gathered Z rows per destination vertex
     (sorted by vertex) -> numerator (256 cols) and denominator (4 cols);
     divide per head.
  5. ELU -> LayerNorm -> GELU -> conv matmul -> X + gamma * Xo.

Sharding: nodes (and pairs grouped by destination vertex) across 8 cores;
weights and edge tables replicated; one AllReduce of esum is the only
collective.
"""

import os

import numpy as np
import ml_dtypes

import concourse.bass as bass
import concourse.bacc as bacc
import concourse.tile as tile
import concourse.mybir as mybir
from concourse.bass_utils import run_bass_kernel_spmd
from concourse.library_config import mlp

BF16 = mybir.dt.bfloat16
F8 = mybir.dt.float8e4
F32 = mybir.dt.float32
I16 = mybir.dt.int16
AL = mybir.AluOpType
AF = mybir.ActivationFunctionType

P = 128
NCORES = 8

N_NODES = 100000
N_EDGES = 20000
NNZ = 500000
CH = 256
H = 4
DH = 64
NEG_SLOPE = 0.2
LN_EPS = 1e-6

NPC = N_NODES // NCORES          # 12500
VG = (NPC + P - 1) // P          # 98
NPC_PAD = VG * P                 # 12544
EG = (N_EDGES + P - 1) // P      # 157
E_PAD = EG * P                   # 20096

ZW = 512                         # Z table row stride in fp8 bytes (260 used)
GATHER_CALL_V2E = 4096
GATHER_CALL_E2V = 4096
SELW = 16                        # indicator chunks built per DVE op


def _bf(x):
    return np.asarray(x, dtype=ml_dtypes.bfloat16)


def _f8(x):
    return np.asarray(x, dtype=ml_dtypes.float8_e4m3)


def _wrap16(idx):
    """dma_gather index layout: index i -> [i % 16, i // 16], replicated x8."""
    assert idx.size % 16 == 0
    w = idx.reshape(-1, 16).T
    return np.ascontiguousarray(np.tile(w, (8, 1))).astype(np.int16)


def _pairmajor(vals, dtype):
    """pair i -> [i % 128, i // 128]."""
    assert vals.size % P == 0
    return np.ascontiguousarray(vals.reshape(-1, P).T).astype(dtype)


def _sel_bytes(rel):
    """rel [M, P] (0..127 valid, 255 pad) -> fp8 one-hot sel tensor
    [P(slot), M*P] with [p, m*128+d] = (rel[m, p] == d)."""
    M = rel.shape[0]
    sel = np.zeros((M, P, P), dtype=ml_dtypes.float8_e4m3)
    m_i, p_i = np.nonzero(rel != 255.0)
    sel[m_i, p_i, rel[m_i, p_i].astype(np.int64)] = 1.0
    return np.ascontiguousarray(sel.transpose(1, 0, 2)).reshape(P, M * P)


def _balance_groups(deg):
    """Assign NPC vertices to VG groups of <=128, balancing pair sums.
    Returns perm: old local id -> new local id."""
    import heapq
    order = np.argsort(-deg, kind="stable")
    heap = [(0, 0, g) for g in range(VG)]
    heapq.heapify(heap)
    perm = np.zeros(NPC, dtype=np.int64)
    fill = np.zeros(VG, dtype=np.int64)
    for v in order:
        while True:
            s, cnt, g = heapq.heappop(heap)
            if cnt < P:
                perm[v] = g * P + fill[g]
                fill[g] += 1
                heapq.heappush(heap, (s + int(deg[v]), cnt + 1, g))
                break
    return perm


def _build_streams_unaligned(vals, rels, cnts):
    """Common unaligned layout: group g occupies slots [S_g, S_g+cap_g),
    cap_g = max over cores. Returns idx [NCORES,T], rel columns per matmul
    entry [NCORES, M, P], and per-group entry chunk lists."""
    ngroups = cnts.shape[1]
    caps = np.maximum(cnts.max(axis=0), 1)
    starts = np.concatenate([[0], np.cumsum(caps)])
    T = int(-(-starts[-1] // P) * P)
    entries = []            # (g, k) in emission order (g-major)
    group_ks = []
    for g in range(ngroups):
        k0 = int(starts[g] // P)
        k1 = int((starts[g + 1] - 1) // P)
        ks = list(range(k0, k1 + 1))
        group_ks.append(ks)
        entries.extend((g, k) for k in ks)
    M = len(entries)
    idx_s = np.zeros((NCORES, T), dtype=np.int64)
    rel_s = np.full((NCORES, M, P), 255.0, dtype=np.float32)
    for c in range(NCORES):
        gstart = np.concatenate([[0], np.cumsum(cnts[c])])
        for g in range(ngroups):
            n = int(cnts[c][g])
            if n == 0:
                continue
            s, d = int(gstart[g]), int(starts[g])
            idx_s[c, d:d + n] = vals[c][s:s + n]
        for m, (g, k) in enumerate(entries):
            lo = max(int(starts[g]), k * P)
            hi = min(int(starts[g]) + int(cnts[c][g]), (k + 1) * P)
            if hi <= lo:
                continue
            s = int(gstart[g]) + (lo - int(starts[g]))
            rel_s[c, m, lo - k * P:hi - k * P] = rels[c][s:s + hi - lo] % P
    return idx_s, rel_s, group_ks, T, M


def make_plan(edge_idx, vertex_idx):
    """Host-side index preprocessing (graph structure only)."""
    edge_idx = np.asarray(edge_idx).astype(np.int64)
    vertex_idx = np.asarray(vertex_idx).astype(np.int64)
    core = vertex_idx // NPC
    lv = vertex_idx - core * NPC

    v2e_e, v2e_lv = [], []
    e2v_e, e2v_nlv = [], []
    perms = []
    for c in range(NCORES):
        m = core == c
        e_c, lv_c = edge_idx[m], lv[m]
        o = np.argsort(e_c, kind="stable")
        v2e_e.append(e_c[o])
        v2e_lv.append(lv_c[o])
        deg = np.bincount(lv_c, minlength=NPC)
        perm = _balance_groups(deg)
        perms.append(perm)
        nlv_c = perm[lv_c]
        o = np.argsort(nlv_c, kind="stable")
        e2v_e.append(e_c[o])
        e2v_nlv.append(nlv_c[o])

    def group_counts(keys_list, ngroups):
        cnts = np.zeros((NCORES, ngroups), dtype=np.int64)
        for c in range(NCORES):
            cnts[c] = np.bincount(keys_list[c] // P, minlength=ngroups)
        return cnts

    v2e_cnts = group_counts(v2e_e, EG)
    e2v_cnts = group_counts(e2v_nlv, VG)

    v2e_idx, v2e_rel, v2e_gks, v2e_T, v2e_M = _build_streams_unaligned(
        v2e_lv, v2e_e, v2e_cnts)
    e2v_idx, e2v_rel, e2v_gks, e2v_T, e2v_M = _build_streams_unaligned(
        e2v_e, e2v_nlv, e2v_cnts)

    cnt = np.bincount(edge_idx, minlength=E_PAD).astype(np.float32)
    inv_cnt = 1.0 / np.maximum(cnt, 1.0)

    return dict(
        v2e_gks=v2e_gks, e2v_gks=e2v_gks,
        v2e_T=v2e_T, e2v_T=e2v_T, v2e_M=v2e_M, e2v_M=e2v_M,
        v2e_idx=v2e_idx, v2e_rel=v2e_rel,
        e2v_idx=e2v_idx, e2v_rel=e2v_rel,
        perms=perms,
        inv_cnt=inv_cnt,
    )


def _n_gather_calls(total_chunks, call_pairs):
    total = total_chunks * P
    n_full, rem = divmod(total, call_pairs)
    sizes = [call_pairs] * n_full
    if rem:
        sizes.append(rem)
    return sizes


def build_kernel(v2e_gks, e2v_gks, v2e_T, e2v_T, v2e_M, e2v_M,
                 debug_tables=False):
    v2e_tot = v2e_T
    e2v_tot = e2v_T

    nc = bacc.Bacc("TRN2", target_bir_lowering=False, debug=False,
                   num_devices=NCORES, num_swdge_queues=2,
                   dynamic_dma_scratch_size=32768)

    x_in = nc.dram_tensor("x", [NPC_PAD, CH], BF16, kind="ExternalInput")
    xt_in = nc.dram_tensor("xt", [P, VG * 2 * P], BF16, kind="ExternalInput")
    wcat_in = nc.dram_tensor("wcat", [CH, CH], BF16, kind="ExternalInput")
    convw_in = nc.dram_tensor("convw", [CH, CH], BF16, kind="ExternalInput")
    browbf_in = nc.dram_tensor("browbf", [1, CH], BF16, kind="ExternalInput")
    awrep_in = nc.dram_tensor("awrep", [P, CH], BF16, kind="ExternalInput")
    convbrep_in = nc.dram_tensor("convbrep", [P, CH], F32, kind="ExternalInput")
    gammarep_in = nc.dram_tensor("gammarep", [P, CH], F32, kind="ExternalInput")
    lnwrep_in = nc.dram_tensor("lnwrep", [P, CH], F32, kind="ExternalInput")
    lnbrep_in = nc.dram_tensor("lnbrep", [P, CH], F32, kind="ExternalInput")
    iota_in = nc.dram_tensor("iota", [P, P], BF16, kind="ExternalInput")
    ident_in = nc.dram_tensor("ident", [P, P], BF16, kind="ExternalInput")
    invc_in = nc.dram_tensor("invc", [P, EG], F32, kind="ExternalInput")
    c14_in = nc.dram_tensor("c14", [P, H], F32, kind="ExternalInput")
    epscol_in = nc.dram_tensor("epscol", [P, 1], F32, kind="ExternalInput")
    v2ei_in = nc.dram_tensor("v2ei", [P, v2e_tot // 16], I16, kind="ExternalInput")
    v2er_in = nc.dram_tensor("v2er", [P, v2e_M], BF16, kind="ExternalInput")
    e2vi_in = nc.dram_tensor("e2vi", [P, e2v_tot // 16], I16, kind="ExternalInput")
    e2vr_in = nc.dram_tensor("e2vr", [P, e2v_M], BF16, kind="ExternalInput")
    # f32: the gamma-scaled (1e-6) GNN term is below bf16 resolution vs X,
    # so a bf16 output would silently drop it.
    out_ext = nc.dram_tensor("out", [NPC_PAD, CH], F32, kind="ExternalOutput")
    dbg = {}
    if debug_tables:
        dbg["xh"] = nc.dram_tensor("dbg_xh", [NPC_PAD, CH], F8, kind="ExternalOutput")
        dbg["esum"] = nc.dram_tensor("dbg_esum", [E_PAD, CH], BF16, kind="ExternalOutput")
        dbg["z"] = nc.dram_tensor("dbg_z", [E_PAD, ZW], F8, kind="ExternalOutput")
        dbg["xn"] = nc.dram_tensor("dbg_xn", [NPC_PAD, CH], BF16, kind="ExternalOutput")

    def rows(dr, t0, w):
        return dr[t0 * P:(t0 + w) * P, :].rearrange("(t p) f -> p t f", p=P)

    with tile.TileContext(nc) as tc:
        with tc.tile_pool(name="dram", bufs=1, space="DRAM") as dram, \
             tc.tile_pool(name="const", bufs=1) as cpool, \
             tc.tile_pool(name="resident", bufs=1) as rpool:

            nc.gpsimd.load_library(mlp)

            xh_table = dram.tile([NPC_PAD, CH], F8)
            esum_bounce = dram.tile([E_PAD, CH], BF16)
            # non-final bounds must be multiples of 4: esum is evacuated in
            # 4-group batches and the AR read races a not-yet-emitted write
            # otherwise.
            AR_BOUNDS = [36, 84, 120, 144, 152, EG]
            _ar_lims = list(zip([0] + AR_BOUNDS[:-1], AR_BOUNDS))
            yfulls = []
            for _ci, (_a, _b) in enumerate(_ar_lims):
                yf = dram.tile([(_b - _a) * P, CH], BF16, addr_space="Shared",
                               name=f"yfull{_ci}", tag=f"yfull{_ci}")
                yfulls.append(yf)
            z_table = dram.tile([E_PAD, ZW], F8)

            def yrows(t0, w):
                """rows [t0*128,(t0+w)*128) of the chunked AR output; the
                caller must not cross an AR chunk boundary."""
                for (_a, _b), yf in zip(_ar_lims, yfulls):
                    if t0 >= _a and t0 + w <= _b:
                        return yf[(t0 - _a) * P:(t0 - _a + w) * P, :].rearrange(
                            "(t p) f -> p t f", p=P)
                raise AssertionError("yrows crosses AR chunk")

            def cload(dr, shape, dtype, name):
                t = cpool.tile(shape, dtype, name=name, tag=name)
                nc.sync.dma_start(t[:], dr[:])
                return t

            w_sb = cpool.tile([P, 2, CH], BF16)
            nc.sync.dma_start(w_sb[:], wcat_in[:].rearrange("(k p) f -> p k f", p=P))
            convw_sb = cpool.tile([P, 2, CH], BF16)
            nc.sync.dma_start(convw_sb[:], convw_in[:].rearrange("(k p) f -> p k f", p=P))
            browbf = cload(browbf_in, [1, CH], BF16, "browbf")
            ones1 = cpool.tile([1, P], BF16)
            nc.vector.memset(ones1[:], 1.0)
            awrep = cload(awrep_in, [P, CH], BF16, "awrep")
            convbrep = cload(convbrep_in, [P, CH], F32, "convbrep")
            gammarep = cload(gammarep_in, [P, CH], F32, "gammarep")
            lnwrep = cload(lnwrep_in, [P, CH], F32, "lnwrep")
            lnbrep = cload(lnbrep_in, [P, CH], F32, "lnbrep")
            iota = cload(iota_in, [P, P], BF16, "iota")
            ident = cload(ident_in, [P, P], BF16, "ident")
            invc = cload(invc_in, [P, EG], F32, "invc")
            c14 = cload(c14_in, [P, H], F32, "c14")
            epscol = cload(epscol_in, [P, 1], F32, "epscol")
            gcb = cpool.tile([P, CH], F32)
            nc.vector.tensor_tensor(out=gcb[:], in0=gammarep[:], in1=convbrep[:],
                                    op=AL.mult)
            # Warmup collective: the first AllReduce pays ~150us of CC mesh
            # init; absorb it here, overlapped with phase 1.
            warm_in = dram.tile([P, CH], BF16)
            warm_out = dram.tile([P, CH], BF16, addr_space="Shared",
                                 name="warmout", tag="warmout")
            wt = cpool.tile([P, CH], BF16)
            nc.vector.memset(wt[:], 0.0)
            nc.sync.dma_start(warm_in[:], wt[:])
            nc.gpsimd.collective_compute(
                "AllReduce", AL.add, replica_groups=[list(range(NCORES))],
                ins=[warm_in[:].opt()], outs=[warm_out[:].opt()])
            # fold gamma into conv_w columns: (Xg @ W) * gamma = Xg @ (W * gamma_row)
            nc.vector.tensor_tensor(
                out=convw_sb[:], in0=convw_sb[:],
                in1=gammarep[:, None, :].to_broadcast([P, 2, CH]), op=AL.mult)

            def z_blocks(zpool, t_lo, t_hi):
                for t0 in range(t_lo, t_hi, 8):
                    w = min(8, t_hi - t0)
                    y4 = zpool.tile([P, 8, CH], BF16, tag="zy")
                    nc.sync.dma_start(y4[:, :w, :], yrows(t0, w))
                    tmp = zpool.tile([P, 8, CH], BF16, tag="ztmp")
                    nc.vector.tensor_tensor(
                        out=tmp[:, :w, :], in0=y4[:, :w, :],
                        in1=awrep[:, None, :].to_broadcast([P, w, CH]), op=AL.mult)
                    beta = zpool.tile([P, 8, H], F32, tag="zbeta")
                    nc.vector.tensor_reduce(
                        out=beta[:, :w, :],
                        in_=tmp[:, :w, :].rearrange("p t (h d) -> p t h d", d=DH),
                        axis=mybir.AxisListType.X, op=AL.add)
                    al_ = zpool.tile([P, 8, H], F32, tag="zal")
                    nc.vector.tensor_tensor(
                        out=al_[:, :w, :], in0=beta[:, :w, :],
                        in1=invc[:, t0:t0 + w, None].to_broadcast([P, w, H]),
                        op=AL.mult)
                    sal = zpool.tile([P, 8, H], F32, tag="zsal")
                    nc.scalar.activation(out=sal[:, :w, :], in_=al_[:, :w, :],
                                         func=AF.Prelu, alpha=NEG_SLOPE)
                    zrow = zpool.tile([P, 8, CH + H], F8, tag="zrow")
                    expS = zrow[:, :w, CH:CH + H]
                    nc.scalar.activation(out=expS, in_=sal[:, :w, :], func=AF.Exp)
                    s4 = zpool.tile([P, 8, H], F32, tag="zs4")
                    nc.vector.tensor_tensor(
                        out=s4[:, :w, :], in0=expS,
                        in1=invc[:, t0:t0 + w, None].to_broadcast([P, w, H]),
                        op=AL.mult)
                    nc.vector.tensor_tensor(
                        out=zrow[:, :w, :CH].rearrange("p t (h d) -> p t h d", d=DH),
                        in0=y4[:, :w, :].rearrange("p t (h d) -> p t h d", d=DH),
                        in1=s4[:, :w, :, None].to_broadcast([P, w, H, DH]),
                        op=AL.mult)
                    nc.sync.dma_start(
                        z_table[t0 * P:(t0 + w) * P, :CH + H].rearrange(
                            "(t p) f -> p t f", p=P),
                        zrow[:, :w, :])


            # ================= Phase 1: Xh = X @ W + b =================
            with tc.tile_pool(name="p1sb", bufs=3) as p1sb, \
                 tc.tile_pool(name="p1xt", bufs=1) as p1xt, \
                 tc.tile_pool(name="p1ps", bufs=2, space="PSUM") as p1ps:
                xt_sb = p1xt.tile([P, VG * 2 * P], BF16, tag="xt")
                XTC = VG * 2 * P // 4
                assert XTC * 4 == VG * 2 * P
                for c4 in range(4):
                    nc.sync.dma_start(xt_sb[:, c4 * XTC:(c4 + 1) * XTC],
                                      xt_in[:, c4 * XTC:(c4 + 1) * XTC])
                xt_v = xt_sb[:].rearrange("p (t k f) -> p t k f", t=VG, k=2)
                xh4, t0, tw = None, 0, 0
                for t in range(VG):
                    psf = p1ps.tile([P, 512], F32, tag="xhps")
                    ps = psf[:, :CH]
                    for k in range(2):
                        nc.tensor.matmul(ps, lhsT=xt_v[:, t, k, :], rhs=w_sb[:, k, :],
                                         start=(k == 0), stop=False)
                    nc.tensor.matmul(ps, lhsT=ones1[:], rhs=browbf[:],
                                     start=False, stop=True)
                    if t % 4 == 0:
                        t0 = t
                        tw = min(4, VG - t0)
                        xh4 = p1sb.tile([P, 4, CH], F8, tag="xhout")
                    nc.scalar.copy(out=xh4[:, t - t0, :], in_=ps)
                    if t - t0 == tw - 1:
                        nc.sync.dma_start(rows(xh_table, t0, tw), xh4[:, :tw, :])
                if debug_tables:
                    nc.sync.dma_start(dbg["xh"][:], xh_table[:])

            # ================= Phase 2: v2e partial esum =================
            with tc.tile_pool(name="v2esb", bufs=4) as gpool, \
                 tc.tile_pool(name="v2esel", bufs=3) as selpool, \
                 tc.tile_pool(name="v2eev", bufs=3) as evpool, \
                 tc.tile_pool(name="v2eidx", bufs=1) as ipool, \
                 tc.tile_pool(name="zsbv", bufs=3) as zpool_v2e, \
                 tc.tile_pool(name="v2eps", bufs=4, space="PSUM") as v2eps:
                v2ei = ipool.tile([P, v2e_tot // 16], I16)
                nc.sync.dma_start(v2ei[:], v2ei_in[:])
                v2er = ipool.tile([P, v2e_M], BF16)
                nc.sync.dma_start(v2er[:], v2er_in[:])
                call_sizes = _n_gather_calls(v2e_tot // P, GATHER_CALL_V2E)
                gtiles = [None] * len(call_sizes)
                sel_cur, sel0 = None, 0
                esb4, e0, ew = None, 0, 0
                ent = 0
                # z_blocks for AR chunk ci run LAG groups after its bound, in
                # fine-grained strips of 8 tiles to avoid DVE bursts.
                LAG = 26
                ZSTRIP = 8
                _zq = {}
                for _ci, (_a, _b) in enumerate(_ar_lims):
                    for _si, _t0 in enumerate(range(_a, _b, ZSTRIP)):
                        _zq.setdefault(_b + LAG + 3 * _si, []).append(
                            (_t0, min(_t0 + ZSTRIP, _b)))
                for g in range(EG):
                    for _a, _b in _zq.get(g, []):
                        z_blocks(zpool_v2e, _a, _b)
                    psf = v2eps.tile([P, 512], F32, tag="v2eps")
                    ps = psf[:, :CH]
                    ks = v2e_gks[g]
                    for i, k in enumerate(ks):
                        gc, j = divmod(k, GATHER_CALL_V2E // P)
                        if gtiles[gc] is None:
                            n = call_sizes[gc]
                            gt = gpool.tile([P, GATHER_CALL_V2E // P, CH], F8,
                                            tag="v2egather")
                            s = gc * GATHER_CALL_V2E
                            nc.gpsimd.dma_gather(
                                gt[:, :n // P, :], xh_table[:],
                                v2ei[:, s // 16:(s + n) // 16], n, n, CH,
                                single_packet=False, queue_num=gc % 2)
                            gtiles[gc] = gt
                        if ent % SELW == 0:
                            sel0 = ent
                            sw = min(SELW, v2e_M - ent)
                            sel_cur = selpool.tile([P, SELW, P], F8, tag="v2esel")
                            nc.vector.tensor_tensor(
                                out=sel_cur[:, :sw, :],
                                in0=v2er[:, ent:ent + sw, None].to_broadcast(
                                    [P, sw, P]),
                                in1=iota[:, None, :].to_broadcast([P, sw, P]),
                                op=AL.is_equal)
                        nc.tensor.matmul(ps, lhsT=sel_cur[:, ent - sel0, :],
                                         rhs=gtiles[gc][:, j, :],
                                         start=(i == 0), stop=(i == len(ks) - 1))
                        ent += 1
                    if g % 4 == 0:
                        e0 = g
                        ew = min(4, EG - e0)
                        esb4 = evpool.tile([P, 4, CH], BF16, tag="v2eev")
                    nc.scalar.copy(out=esb4[:, g - e0, :], in_=ps)
                    if g - e0 == ew - 1:
                        nc.sync.dma_start(rows(esum_bounce, e0, ew), esb4[:, :ew, :])
                    if g + 1 in AR_BOUNDS:
                        ci = AR_BOUNDS.index(g + 1)
                        a = 0 if ci == 0 else AR_BOUNDS[ci - 1]
                        nc.gpsimd.collective_compute(
                            "AllReduce", AL.add,
                            replica_groups=[list(range(NCORES))],
                            ins=[esum_bounce[a * P:(g + 1) * P, :].opt()],
                            outs=[yfulls[ci].opt()])

                for g in range(EG, EG + 96):
                    for _a, _b in _zq.get(g, []):
                        z_blocks(zpool_v2e, _a, _b)

            # ================= Phase 3: AllReduce (issued chunked in phase 2)
            if debug_tables:
                for (_a, _b), yf in zip(_ar_lims, yfulls):
                    nc.sync.dma_start(dbg["esum"][_a * P:_b * P, :], yf[:])

            if debug_tables:
                with tc.tile_pool(name="dbgz", bufs=1) as _dzp:
                    nc.sync.dma_start(dbg["z"][:], z_table[:])

            # ================= Phase 5: e2v + ELU + LN =================
            with tc.tile_pool(name="e2vsb", bufs=3) as gpool2, \
                 tc.tile_pool(name="e2vsel", bufs=3) as selpool2, \
                 tc.tile_pool(name="e2vev", bufs=2) as evpool2, \
                 tc.tile_pool(name="e2vidx", bufs=1) as ipool2, \
                 tc.tile_pool(name="fsb", bufs=3) as fpool, \
                 tc.tile_pool(name="fps", bufs=2, space="PSUM") as fps, \
                 tc.tile_pool(name="ftps", bufs=2, space="PSUM") as ftps, \
                 tc.tile_pool(name="e2vps", bufs=3, space="PSUM") as e2vps:
                e2vi = ipool2.tile([P, e2v_tot // 16], I16)
                nc.sync.dma_start(e2vi[:], e2vi_in[:])
                e2vr = ipool2.tile([P, e2v_M], BF16)
                nc.sync.dma_start(e2vr[:], e2vr_in[:])
                call_sizes = _n_gather_calls(e2v_tot // P, GATHER_CALL_E2V)
                gtiles = [None] * len(call_sizes)
                sel_cur, sel0 = None, 0
                elu4, l0, lw = None, 0, 0
                ent = 0
                xc_buf = ipool2.tile([P, VG, CH], BF16)
                var_buf = ipool2.tile([P, VG], F32)
                _PB_BOUNDS = [16, 32, 48, 64, 80, 84, 88, 92, 96]

                def pass_b(lo, hi):
                    vb = evpool2.tile([P, 16], F32, tag="vbeps")
                    nc.vector.tensor_scalar_add(vb[:, :hi - lo],
                                                var_buf[:, lo:hi], LN_EPS)
                    vrec = evpool2.tile([P, 16], F32, tag="vrec")
                    nc.vector.reciprocal(vrec[:, :hi - lo], vb[:, :hi - lo])
                    rstd = evpool2.tile([P, 16], F32, tag="rstd")
                    nc.scalar.activation(out=rstd[:, :hi - lo],
                                         in_=vrec[:, :hi - lo], func=AF.Sqrt)
                    for b0 in range(lo, hi, 4):
                        bw = min(4, hi - b0)
                        t4 = evpool2.tile([P, 4, CH], BF16, tag="t4")
                        nc.vector.tensor_tensor(
                            out=t4[:, :bw, :], in0=xc_buf[:, b0:b0 + bw, :],
                            in1=rstd[:, b0 - lo:b0 - lo + bw, None].to_broadcast(
                                [P, bw, CH]),
                            op=AL.mult)
                        t5 = evpool2.tile([P, 4, CH], F32, tag="t5")
                        nc.vector.tensor_tensor(
                            out=t5[:, :bw, :], in0=t4[:, :bw, :],
                            in1=lnwrep[:, None, :].to_broadcast([P, bw, CH]),
                            op=AL.mult)
                        xnb = evpool2.tile([P, 4, CH], BF16, tag="xnb")
                        nc.vector.tensor_tensor(
                            out=xnb[:, :bw, :], in0=t5[:, :bw, :],
                            in1=lnbrep[:, None, :].to_broadcast([P, bw, CH]),
                            op=AL.add)
                        if debug_tables:
                            nc.sync.dma_start(rows(dbg["xn"], b0, bw),
                                              xnb[:, :bw, :])
                        xg4 = fpool.tile([P, 4, CH], BF16, tag="xg4")
                        nc.scalar.activation(out=xg4[:, :bw, :], in_=xnb[:, :bw, :],
                                             func=AF.Gelu)
                        x4 = fpool.tile([P, 4, CH], BF16, tag="x4")
                        nc.sync.dma_start(x4[:, :bw, :], rows(x_in, b0, bw))
                        xgc4 = fpool.tile([P, 4, CH], F32, tag="xgc4")
                        nc.vector.tensor_tensor(
                            out=xgc4[:, :bw, :], in0=x4[:, :bw, :],
                            in1=gcb[:, None, :].to_broadcast([P, bw, CH]),
                            op=AL.add)
                        ofin4 = fpool.tile([P, 4, CH], F32, tag="ofin4")
                        for j in range(bw):
                            xgT = fpool.tile([P, 2, P], BF16, tag="xgT")
                            for k in range(2):
                                tp = ftps.tile([P, P], BF16, tag="tps")
                                nc.tensor.transpose(tp[:],
                                                    xg4[:, j, k * P:(k + 1) * P],
                                                    ident[:])
                                nc.scalar.copy(out=xgT[:, k, :], in_=tp[:])
                            psf2 = fps.tile([P, 512], F32, tag="fps")
                            ps2 = psf2[:, :CH]
                            for k in range(2):
                                nc.tensor.matmul(ps2, lhsT=xgT[:, k, :],
                                                 rhs=convw_sb[:, k, :],
                                                 start=(k == 0), stop=(k == 1))
                            nc.vector.tensor_tensor(out=ofin4[:, j, :], in0=ps2,
                                                    in1=xgc4[:, j, :], op=AL.add)
                        nc.sync.dma_start(rows(out_ext, b0, bw), ofin4[:, :bw, :])

                for g in range(VG):
                    psf = e2vps.tile([P, 512], F32, tag="e2vps")
                    ps = psf[:, :CH + H]
                    ks = e2v_gks[g]
                    for i, k in enumerate(ks):
                        gc, j = divmod(k, GATHER_CALL_E2V // P)
                        if gtiles[gc] is None:
                            n = call_sizes[gc]
                            gt = gpool2.tile([P, GATHER_CALL_E2V // P, ZW], F8,
                                             tag="e2vgather")
                            s = gc * GATHER_CALL_E2V
                            nc.gpsimd.dma_gather(
                                gt[:, :n // P, :], z_table[:],
                                e2vi[:, s // 16:(s + n) // 16], n, n, ZW,
                                single_packet=False, queue_num=gc % 2)
                            gtiles[gc] = gt
                        if ent % SELW == 0:
                            sel0 = ent
                            sw = min(SELW, e2v_M - ent)
                            sel_cur = selpool2.tile([P, SELW, P], F8, tag="e2vsel")
                            nc.vector.tensor_tensor(
                                out=sel_cur[:, :sw, :],
                                in0=e2vr[:, ent:ent + sw, None].to_broadcast(
                                    [P, sw, P]),
                                in1=iota[:, None, :].to_broadcast([P, sw, P]),
                                op=AL.is_equal)
                        nc.tensor.matmul(ps, lhsT=sel_cur[:, ent - sel0, :],
                                         rhs=gtiles[gc][:, j, :CH + H],
                                         start=(i == 0), stop=(i == len(ks) - 1))
                        ent += 1
                    # xpre = num/den ; ELU = exp(min(x,0)) - 1 + relu(x)
                    den = evpool2.tile([P, H], F32, tag="den")
                    nc.vector.tensor_scalar_max(den[:], ps[:, CH:CH + H], 1e-12)
                    rec = evpool2.tile([P, H], F32, tag="rec")
                    nc.vector.reciprocal(rec[:], den[:])
                    xpre = evpool2.tile([P, CH], BF16, tag="xpre")
                    nc.vector.tensor_tensor(
                        out=xpre[:].rearrange("p (h d) -> p h d", d=DH),
                        in0=ps[:, :CH].rearrange("p (h d) -> p h d", d=DH),
                        in1=rec[:, :, None].to_broadcast([P, H, DH]),
                        op=AL.mult)
                    relx = evpool2.tile([P, CH], BF16, tag="relx")
                    nc.scalar.activation(out=relx[:], in_=xpre[:], func=AF.Relu)
                    m0 = evpool2.tile([P, CH], BF16, tag="m0")
                    nc.scalar.activation(out=m0[:], in_=xpre[:], func=AF.Relu,
                                         scale=-1.0)
                    ep = evpool2.tile([P, CH], F32, tag="ep")
                    nc.scalar.activation(out=ep[:], in_=m0[:], func=AF.Exp,
                                         scale=-1.0)
                    if g % 4 == 0:
                        l0 = g
                        lw = min(4, VG - l0)
                        elu4 = evpool2.tile([P, 4, CH], BF16, tag="elu4")
                    nc.vector.scalar_tensor_tensor(
                        out=elu4[:, g - l0, :], in0=ep[:], scalar=-1.0, in1=relx[:],
                        op0=AL.add, op1=AL.add)
                    if g - l0 == lw - 1:
                        mu4 = evpool2.tile([P, 4], F32, tag="mu4")
                        nc.vector.tensor_reduce(out=mu4[:, :lw], in_=elu4[:, :lw, :],
                                                axis=mybir.AxisListType.X, op=AL.add)
                        nc.vector.tensor_tensor(out=mu4[:, :lw], in0=mu4[:, :lw],
                                                in1=c14[:, :lw], op=AL.mult)
                        xc4 = xc_buf[:, l0:l0 + lw, :]
                        nc.vector.tensor_tensor(
                            out=xc4, in0=elu4[:, :lw, :],
                            in1=mu4[:, :lw, None].to_broadcast([P, lw, CH]),
                            op=AL.subtract)
                        sq4 = evpool2.tile([P, 4, CH], BF16, tag="sq4")
                        nc.vector.tensor_tensor(out=sq4[:, :lw, :], in0=xc4,
                                                in1=xc4, op=AL.mult)
                        ss4 = evpool2.tile([P, 4], F32, tag="ss4")
                        nc.vector.tensor_reduce(out=ss4[:, :lw], in_=sq4[:, :lw, :],
                                                axis=mybir.AxisListType.X, op=AL.add)
                        nc.vector.tensor_tensor(out=var_buf[:, l0:l0 + lw],
                                                in0=ss4[:, :lw],
                                                in1=c14[:, :lw], op=AL.mult)
                    if (g + 1) in _PB_BOUNDS:
                        pi = _PB_BOUNDS.index(g + 1)
                        pass_b(0 if pi == 0 else _PB_BOUNDS[pi - 1], g + 1)
                if _PB_BOUNDS[-1] < VG:
                    pass_b(_PB_BOUNDS[-1], VG)

    nc.compile()
    return nc


def prepare_inputs(X, edge_idx, vertex_idx, theta_w, theta_b, atten_w,
                   ln_w, ln_b, conv_w, conv_b, gamma, plan):
    X = np.asarray(X, dtype=np.float32)
    theta_w = np.asarray(theta_w, dtype=np.float32)
    wcat = _bf(theta_w.transpose(1, 0, 2).reshape(CH, CH))
    browbf = _bf(np.asarray(theta_b, np.float32).reshape(1, CH))
    awrep = _bf(np.tile(np.asarray(atten_w, np.float32).reshape(1, CH), (P, 1)))
    convw = _bf(np.asarray(conv_w, np.float32))
    convbrep = np.tile(np.asarray(conv_b, np.float32).reshape(1, CH), (P, 1))
    gammarep = np.tile(np.asarray(gamma, np.float32).reshape(1, CH), (P, 1))
    lnwrep = np.tile(np.asarray(ln_w, np.float32).reshape(1, CH), (P, 1))
    lnbrep = np.tile(np.asarray(ln_b, np.float32).reshape(1, CH), (P, 1))
    iota = _bf(np.tile(np.arange(P, dtype=np.float32), (P, 1)))
    ident = _bf(np.eye(P, dtype=np.float32))
    invc = np.ascontiguousarray(
        plan["inv_cnt"].reshape(EG, P).T).astype(np.float32)
    c14 = np.full((P, H), 1.0 / CH, np.float32)
    epscol = np.full((P, 1), LN_EPS, np.float32)

    in_maps = []
    for c in range(NCORES):
        xc = np.zeros((NPC_PAD, CH), np.float32)
        xc[:NPC] = X[c * NPC:(c + 1) * NPC]
        xcb = _bf(xc)
        # xt / xh_table stay in original local order (v2e gathers by old lv);
        # the residual input x follows the e2v output renumbering.
        xt = np.ascontiguousarray(
            _bf(xc).reshape(VG, P, 2, P).transpose(3, 0, 2, 1)).reshape(
                P, VG * 2 * P)
        perm = plan["perms"][c]
        xp = np.zeros((NPC_PAD, CH), np.float32)
        xp[perm] = xc[:NPC]
        in_maps.append(dict(
            x=_bf(xp), xt=xt, wcat=wcat, convw=convw,
            browbf=browbf, awrep=awrep,
            convbrep=convbrep.astype(np.float32),
            gammarep=gammarep.astype(np.float32),
            lnwrep=lnwrep.astype(np.float32), lnbrep=lnbrep.astype(np.float32),
            iota=iota, ident=ident, invc=invc, c14=c14, epscol=epscol,
            v2ei=_wrap16(plan["v2e_idx"][c]),
            v2er=np.ascontiguousarray(
                plan["v2e_rel"][c].T).astype(ml_dtypes.bfloat16),
            e2vi=_wrap16(plan["e2v_idx"][c]),
            e2vr=np.ascontiguousarray(
                plan["e2v_rel"][c].T).astype(ml_dtypes.bfloat16),
        ))
    return in_maps


_CACHE = {}


def kernel(X, edge_idx, vertex_idx, theta_w, theta_b, atten_w,
           ln_w, ln_b, conv_w, conv_b, gamma):
    debug_tables = bool(int(os.environ.get("GNN_DEBUG_TABLES", "0")))
    trace = bool(int(os.environ.get("GNN_TRACE", "0")))

    plan = make_plan(edge_idx, vertex_idx)
    key = (tuple(map(tuple, plan["v2e_gks"])), tuple(map(tuple, plan["e2v_gks"])),
           debug_tables)
    if key not in _CACHE:
        _CACHE[key] = build_kernel(plan["v2e_gks"], plan["e2v_gks"],
                                   plan["v2e_T"], plan["e2v_T"],
                                   plan["v2e_M"], plan["e2v_M"],
                                   debug_tables=debug_tables)
    nc = _CACHE[key]

    in_maps = prepare_inputs(X, edge_idx, vertex_idx, theta_w, theta_b,
                             atten_w, ln_w, ln_b, conv_w, conv_b, gamma, plan)
    res = run_bass_kernel_spmd(nc, in_maps, core_ids=list(range(NCORES)),
                               trace=trace)
    kernel.last_results = res
    outs = []
    for c in range(NCORES):
        o = np.asarray(res.results[c]["out"]).astype(np.float32)
        outs.append(o[plan["perms"][c]])
    return np.concatenate(outs, axis=0)

